# revision 41
# baseline (speedup 1.0000x reference)
"""Trainium2 Bass kernel for the ASAP dual-branch GNN (GraphConv mean-aggr).

Strategy (data-parallel over graphs, 32 graphs per NeuronCore):
  * Host folds each graph's edge list into a dense normalized adjacency
    An[src, dst] = count(src->dst) / max(deg_dst, 1) in bf16 (one bincount
    over all edges), so on-device the GraphConv layers are pure dense
    matmuls: h = relu( wrel^T (x An) + wroot^T x + brel ).
  * Graphs processed in pairs packed side-by-side in the free axis, so the
    shared-weight root matmuls stream 400 columns at once; each pair's
    x + An arrive in ONE 400KB blob DMA issued from the (otherwise idle)
    GpSimd queue.
  * An split into two 100-row src halves serving as the PE moving operand
    with the node-major ys = x@wrel halves as stationary weights.
  * Engine balance: PE matmuls; ACT pair-wide relu+bias and half the
    psum->bf16 copies; DVE the other copies + per-graph pooled readout via
    free-axis tensor_reduce.
  * Small MLP head + log_softmax computed per-core in f32; no collectives.
Host side only does sharding/layout: adjacency histogram + degree fold,
transposes, dtype casts, blob packing, and constant folding of the
mean-pool 1/200 into lin1_w.
"""

import os
import sys

import numpy as np

if "/opt/trn_rl_repo" not in sys.path:
    sys.path.insert(0, "/opt/trn_rl_repo")

B, N, EPG = 256, 200, 3200
F, H, C = 200, 128, 2
NCORES = 8
GPC = B // NCORES  # graphs per core
NPAIR = GPC // 2
NQ = 100  # src-half width

_CACHE = {}


def _f32(x):
    return np.ascontiguousarray(x, dtype=np.float32)


def _build(gpc=GPC, repeat=1):
    import concourse.bass as bass
    import concourse.tile as tile
    from concourse import bacc, mybir

    dt = mybir.dt
    AF = mybir.ActivationFunctionType
    OP = mybir.AluOpType
    assert gpc % 2 == 0
    npair = gpc // 2

    nc = bacc.Bacc("TRN2", target_bir_lowering=False, debug=False)

    # ---- DRAM I/O (graph pairs packed in the free axis, branch inside) ----
    xta_d = nc.dram_tensor("xta", [npair, 128, 2, 2, N], dt.bfloat16, kind="ExternalInput").ap()
    xtb_d = nc.dram_tensor("xtb", [npair, F - 128, 2, 2, N], dt.bfloat16, kind="ExternalInput").ap()
    an_d = nc.dram_tensor("an", [npair, NQ, 2, 2, 2, N], dt.bfloat16, kind="ExternalInput").ap()
    wa_d = nc.dram_tensor("wa", [128, 2, 2, H], dt.bfloat16, kind="ExternalInput").ap()
    wb_d = nc.dram_tensor("wb", [F - 128, 2, 2, H], dt.bfloat16, kind="ExternalInput").ap()
    w2_d = nc.dram_tensor("w2", [128, 2, 2, H], dt.bfloat16, kind="ExternalInput").ap()
    br_d = nc.dram_tensor("brel", [128, 2, 2, 1], dt.float32, kind="ExternalInput").ap()
    l1w_d = nc.dram_tensor("l1w", [128, 4, H], dt.float32, kind="ExternalInput").ap()
    l1b_d = nc.dram_tensor("l1b", [1, H], dt.float32, kind="ExternalInput").ap()
    l2w_d = nc.dram_tensor("l2w", [H, H // 2], dt.float32, kind="ExternalInput").ap()
    l2b_d = nc.dram_tensor("l2b", [1, H // 2], dt.float32, kind="ExternalInput").ap()
    l3w_d = nc.dram_tensor("l3w", [H // 2, C], dt.float32, kind="ExternalInput").ap()
    l3b_d = nc.dram_tensor("l3b", [1, C], dt.float32, kind="ExternalInput").ap()
    out_d = nc.dram_tensor("out", [gpc, C], dt.float32, kind="ExternalOutput").ap()

    # ---- inline constants ----
    onesrow_f_d = nc.inline_tensor(_f32(np.ones((1, 128), np.float32)), "onesrowf").ap()
    ident_d = nc.inline_tensor(_f32(np.eye(128, dtype=np.float32)), "identf").ap()

    with tile.TileContext(nc) as tc:
        with (
            tc.tile_pool(name="cpool", bufs=1) as cpool,
            tc.tile_pool(name="xpool", bufs=4) as xpool,
            tc.tile_pool(name="apool", bufs=4) as apool,
            tc.tile_pool(name="hpool", bufs=6) as hpool,
            tc.tile_pool(name="spool", bufs=12) as spool,
            tc.tile_pool(name="rpool", bufs=4) as rpool,
            tc.tile_pool(name="psC", bufs=4, space="PSUM") as psC,
            tc.tile_pool(name="psD", bufs=4, space="PSUM") as psD,
        ):
            # ---- load conv weights (sync queue, first so conv starts ASAP) ----
            wa = cpool.tile([128, 2, 2, H], dt.bfloat16)
            nc.sync.dma_start(out=wa[:], in_=wa_d[:])
            wb = cpool.tile([F - 128, 2, 2, H], dt.bfloat16)
            nc.sync.dma_start(out=wb[:], in_=wb_d[:])
            w2 = cpool.tile([128, 2, 2, H], dt.bfloat16)
            nc.sync.dma_start(out=w2[:], in_=w2_d[:])
            brl = cpool.tile([128, 2, 2, 1], dt.float32)
            nc.sync.dma_start(out=brl[:], in_=br_d[:])
            # ---- MLP weights on the scalar queue (off the conv-critical path) ----
            onesrow_f = cpool.tile([1, 128], dt.float32)
            nc.scalar.dma_start(out=onesrow_f[:], in_=onesrow_f_d[:])
            ident = cpool.tile([128, 128], dt.float32)
            nc.scalar.dma_start(out=ident[:], in_=ident_d[:])
            l1w = cpool.tile([128, 4, H], dt.float32)
            nc.scalar.dma_start(out=l1w[:], in_=l1w_d[:])
            l1b = cpool.tile([1, H], dt.float32)
            nc.scalar.dma_start(out=l1b[:], in_=l1b_d[:])
            l2w = cpool.tile([H, H // 2], dt.float32)
            nc.scalar.dma_start(out=l2w[:], in_=l2w_d[:])
            l2b = cpool.tile([1, H // 2], dt.float32)
            nc.scalar.dma_start(out=l2b[:], in_=l2b_d[:])
            l3w = cpool.tile([H // 2, C], dt.float32)
            nc.scalar.dma_start(out=l3w[:], in_=l3w_d[:])
            l3b = cpool.tile([1, C], dt.float32)
            nc.scalar.dma_start(out=l3b[:], in_=l3b_d[:])
            # preload exp/ln ACT tables so the log_softmax tail doesn't pay them
            warm = rpool.tile([1, 1], dt.float32, tag="warm")
            nc.scalar.activation(warm[:], onesrow_f[:, 0:1], AF.Exp, bias=0.0, scale=1.0)
            nc.scalar.activation(warm[:], warm[:], AF.Ln, bias=0.0, scale=1.0)

            pooled = [
                [cpool.tile([128, gpc], dt.float32, tag=f"pool{b}{l}", name=f"pooled{b}{l}") for l in range(2)]
                for b in range(2)
            ]

            def conv_pair(b, p, xta, xtb, A):
                # Layer 1: ys = x @ wrel (node-major halves), per graph
                yss = []
                for g in range(2):
                    yr = psC.tile([NQ, 2, H], dt.float32, tag="yr", name=f"yr{b}{p}{g}")
                    for m in range(2):
                        nc.tensor.matmul(
                            yr[:, m, :], lhsT=xta[:, g, m * NQ : (m + 1) * NQ], rhs=wa[:, b, 0, :],
                            start=True, stop=False,
                        )
                        nc.tensor.matmul(
                            yr[:, m, :], lhsT=xtb[:, g, m * NQ : (m + 1) * NQ], rhs=wb[:, b, 0, :],
                            start=False, stop=True,
                        )
                    ys = spool.tile([NQ, 2, H], dt.bfloat16, tag="ys", name=f"ys{b}{p}{g}")
                    if g == 0:
                        nc.vector.tensor_copy(out=ys[:], in_=yr[:])
                    else:
                        nc.scalar.copy(out=ys[:], in_=yr[:])
                    yss.append(ys)

                hp = psD.tile([128, 2, N], dt.float32, tag="hD", name=f"hp{b}{p}")
                nc.tensor.matmul(hp[:, :, :], lhsT=wa[:, b, 1, :], rhs=xta[:, :, :], start=True, stop=False)
                nc.tensor.matmul(hp[:, :, :], lhsT=wb[:, b, 1, :], rhs=xtb[:, :, :], start=False, stop=False)
                for g in range(2):
                    nc.tensor.matmul(hp[:, g, :], lhsT=yss[g][:, 0, :], rhs=A[:, g, 0, :], start=False, stop=False)
                    nc.tensor.matmul(
                        hp[:, g, :], lhsT=yss[g][:, 1, :], rhs=A[:, g, 1, :],
                        start=False, stop=(g == 1),
                    )
                h1 = hpool.tile([128, 2, N], dt.bfloat16, tag="h1", name=f"h1{b}{p}")
                nc.scalar.activation(h1[:, :, :], hp[:, :, :], AF.Relu, bias=brl[:, b, 0, :], scale=1.0)
                nc.vector.tensor_reduce(
                    out=pooled[b][0][:, 2 * p : 2 * p + 2], in_=h1[:, :, :],
                    axis=mybir.AxisListType.X, op=OP.add,
                )

                # Layer 2
                zss = []
                for g in range(2):
                    zr = psC.tile([NQ, 2, H], dt.float32, tag="yr", name=f"zr{b}{p}{g}")
                    for m in range(2):
                        nc.tensor.matmul(
                            zr[:, m, :], lhsT=h1[:, g, m * NQ : (m + 1) * NQ], rhs=w2[:, b, 0, :],
                            start=True, stop=True,
                        )
                    zs = spool.tile([NQ, 2, H], dt.bfloat16, tag="ys", name=f"zs{b}{p}{g}")
                    if g == 0:
                        nc.vector.tensor_copy(out=zs[:], in_=zr[:])
                    else:
                        nc.scalar.copy(out=zs[:], in_=zr[:])
                    zss.append(zs)

                gp = psD.tile([128, 2, N], dt.float32, tag="hD", name=f"gp{b}{p}")
                nc.tensor.matmul(gp[:, :, :], lhsT=w2[:, b, 1, :], rhs=h1[:, :, :], start=True, stop=False)
                for g in range(2):
                    nc.tensor.matmul(gp[:, g, :], lhsT=zss[g][:, 0, :], rhs=A[:, g, 0, :], start=False, stop=False)
                    nc.tensor.matmul(
                        gp[:, g, :], lhsT=zss[g][:, 1, :], rhs=A[:, g, 1, :],
                        start=False, stop=(g == 1),
                    )
                g1 = hpool.tile([128, 2, N], dt.bfloat16, tag="g1", name=f"g1{b}{p}")
                nc.scalar.activation(g1[:, :, :], gp[:, :, :], AF.Relu, bias=brl[:, b, 1, :], scale=1.0)
                nc.vector.tensor_reduce(
                    out=pooled[b][1][:, 2 * p : 2 * p + 2], in_=g1[:, :, :],
                    axis=mybir.AxisListType.X, op=OP.add,
                )

            # ---- main loop: graph pairs ----
            for _rep in range(repeat):
                for p in range(npair):
                    xtap = xpool.tile([128, 2, 2, N], dt.bfloat16, tag="xta", name=f"xta{p}")
                    nc.gpsimd.dma_start(out=xtap[:], in_=xta_d[p])
                    xtbp = xpool.tile([F - 128, 2, 2, N], dt.bfloat16, tag="xtb", name=f"xtb{p}")
                    nc.gpsimd.dma_start(out=xtbp[:], in_=xtb_d[p])
                    Ap = apool.tile([NQ, 2, 2, 2, N], dt.bfloat16, tag="A", name=f"A{p}")
                    nc.sync.dma_start(out=Ap[:], in_=an_d[p])
                    for b in range(2):
                        conv_pair(b, p, xtap[:, b], xtbp[:, b], Ap[:, b])

            # ---- MLP head (f32) ----
            z1p = psD.tile([gpc, H], dt.float32, tag="hD")
            order = [pooled[0][0], pooled[0][1], pooled[1][0], pooled[1][1]]
            for k in range(4):
                nc.tensor.matmul(z1p[:], lhsT=order[k][:], rhs=l1w[:, k, :], start=(k == 0), stop=False)
            nc.tensor.matmul(z1p[:], lhsT=onesrow_f[:, 0:gpc], rhs=l1b[:], start=False, stop=True)
            z1s = rpool.tile([gpc, H], dt.float32, tag="z1s")
            nc.scalar.activation(z1s[:], z1p[:], AF.Relu, bias=0.0, scale=1.0)

            z1tp = psD.tile([H, gpc], dt.float32, tag="hD")
            nc.tensor.transpose(out=z1tp[:], in_=z1s[:], identity=ident[0:gpc, 0:gpc])
            z1t = rpool.tile([H, gpc], dt.float32, tag="z1t")
            nc.vector.tensor_copy(out=z1t[:], in_=z1tp[:])

            z2p = psD.tile([gpc, H // 2], dt.float32, tag="hD")
            nc.tensor.matmul(z2p[:], lhsT=z1t[:], rhs=l2w[:], start=True, stop=False)
            nc.tensor.matmul(z2p[:], lhsT=onesrow_f[:, 0:gpc], rhs=l2b[:], start=False, stop=True)
            z2s = rpool.tile([gpc, H // 2], dt.float32, tag="z2s")
            nc.scalar.activation(z2s[:], z2p[:], AF.Relu, bias=0.0, scale=1.0)

            z2tp = psD.tile([H // 2, gpc], dt.float32, tag="hD")
            nc.tensor.transpose(out=z2tp[:], in_=z2s[:], identity=ident[0:gpc, 0:gpc])
            z2t = rpool.tile([H // 2, gpc], dt.float32, tag="z2t")
            nc.vector.tensor_copy(out=z2t[:], in_=z2tp[:])

            z3p = psD.tile([gpc, C], dt.float32, tag="hD")
            nc.tensor.matmul(z3p[:], lhsT=z2t[:], rhs=l3w[:], start=True, stop=False)
            nc.tensor.matmul(z3p[:], lhsT=onesrow_f[:, 0:gpc], rhs=l3b[:], start=False, stop=True)

            m = rpool.tile([gpc, 1], dt.float32, tag="lsm")
            nc.vector.tensor_reduce(out=m[:], in_=z3p[:], axis=mybir.AxisListType.X, op=OP.max)
            negm = rpool.tile([gpc, 1], dt.float32, tag="lsnm")
            nc.vector.tensor_scalar(negm[:], m[:], -1.0, None, OP.mult)
            esc = rpool.tile([gpc, C], dt.float32, tag="lse")
            sume = rpool.tile([gpc, 1], dt.float32, tag="lssum")
            nc.scalar.activation(esc[:], z3p[:], AF.Exp, bias=negm[:], scale=1.0, accum_out=sume[:])
            lse = rpool.tile([gpc, 1], dt.float32, tag="lsl")
            nc.scalar.activation(lse[:], sume[:], AF.Ln, bias=0.0, scale=1.0)
            outv = rpool.tile([gpc, C], dt.float32, tag="outv")
            nc.vector.tensor_scalar(outv[:], z3p[:], negm[:], lse[:], OP.add, OP.subtract)
            nc.sync.dma_start(out=out_d[:], in_=outv[:])

    nc.compile()
    return nc


def _prep_inputs(sc_x, fc_x, sc_edge_index, fc_edge_index,
                 sc1_wrel, sc1_brel, sc1_wroot, sc2_wrel, sc2_brel, sc2_wroot,
                 fc1_wrel, fc1_brel, fc1_wroot, fc2_wrel, fc2_brel, fc2_wroot,
                 lin1_w, lin1_b, lin2_w, lin2_b, lin3_w, lin3_b, batch=None):
    import ml_dtypes

    bf = ml_dtypes.bfloat16

    def prep_x(x):
        # [B*N, F] -> [B/2, F, 2, N] (graph pairs packed in free axis)
        return np.asarray(x, np.float32).reshape(B // 2, 2, N, F).transpose(0, 3, 1, 2)

    def prep_A(ei):
        # dense normalized adjacency: An[g, src, dst] = count / max(deg_dst, 1)
        ei = np.asarray(ei).astype(np.int64)
        gid = np.arange(B * EPG, dtype=np.int64) // EPG
        src = ei[0] - gid * N
        dst = ei[1] - gid * N
        flat = (gid * N + src) * N + dst
        cnt = np.bincount(flat, minlength=B * N * N).astype(np.float32).reshape(B, N, N)
        deg = cnt.sum(axis=1)  # in-degree per dst
        An = cnt / np.maximum(deg, 1.0)[:, None, :]
        # [B, N(src), N(dst)] -> [B/2, 2(graph), 2(src half), 100, 200]
        return An.reshape(B // 2, 2, 2, NQ, N)

    # [B/2, F, 2(branch), 2(graph), N]
    xt = np.stack([prep_x(sc_x), prep_x(fc_x)]).transpose(1, 2, 0, 3, 4)
    xta = np.ascontiguousarray(xt[:, 0:128]).astype(bf)
    xtb = np.ascontiguousarray(xt[:, 128:F]).astype(bf)
    # [B/2, 100, 2(branch), 2(graph), 2(half), 200]
    an = np.ascontiguousarray(
        np.stack([prep_A(sc_edge_index), prep_A(fc_edge_index)]).transpose(1, 4, 0, 2, 3, 5)
    ).astype(bf)

    wa = np.stack([
        np.stack([np.asarray(sc1_wrel)[:128], np.asarray(sc1_wroot)[:128]]),
        np.stack([np.asarray(fc1_wrel)[:128], np.asarray(fc1_wroot)[:128]]),
    ]).transpose(2, 0, 1, 3).astype(bf)  # [128, 2, 2, H] (partition-major)
    wb = np.stack([
        np.stack([np.asarray(sc1_wrel)[128:], np.asarray(sc1_wroot)[128:]]),
        np.stack([np.asarray(fc1_wrel)[128:], np.asarray(fc1_wroot)[128:]]),
    ]).transpose(2, 0, 1, 3).astype(bf)  # [72, 2, 2, H]
    w2 = np.stack([
        np.stack([np.asarray(sc2_wrel), np.asarray(sc2_wroot)]),
        np.stack([np.asarray(fc2_wrel), np.asarray(fc2_wroot)]),
    ]).transpose(2, 0, 1, 3).astype(bf)  # [128, 2, 2, H]
    brel = np.stack([
        np.stack([np.asarray(sc1_brel), np.asarray(sc2_brel)]),
        np.stack([np.asarray(fc1_brel), np.asarray(fc2_brel)]),
    ]).astype(np.float32).transpose(2, 0, 1)[:, :, :, None]  # [128, 2, 2, 1]

    l1w = np.asarray(lin1_w, np.float32).copy()
    l1w[:256] *= 1.0 / N  # fold mean-pool divisor for the SC branch readouts
    l1w = np.ascontiguousarray(l1w.reshape(4, 128, H).transpose(1, 0, 2))  # [128, 4, H]

    return dict(
        xta=xta, xtb=xtb, an=an, wa=_c(wa), wb=_c(wb), w2=_c(w2), brel=_c(brel),
        l1w=l1w, l1b=_f32(lin1_b)[None, :], l2w=_f32(lin2_w), l2b=_f32(lin2_b)[None, :],
        l3w=_f32(lin3_w), l3b=_f32(lin3_b)[None, :],
    )


def _c(x):
    return np.ascontiguousarray(x)


def _make_in_maps(full):
    in_maps = []
    for c in range(NCORES):
        ps = slice(c * NPAIR, (c + 1) * NPAIR)
        m = dict(full)
        m["xta"] = np.ascontiguousarray(full["xta"][ps])
        m["xtb"] = np.ascontiguousarray(full["xtb"][ps])
        m["an"] = np.ascontiguousarray(full["an"][ps])
        in_maps.append(m)
    return in_maps


def kernel(**inputs):
    from concourse import bass_utils

    if "nc" not in _CACHE:
        _CACHE["nc"] = _build()
    nc = _CACHE["nc"]

    full = _prep_inputs(**inputs)
    in_maps = _make_in_maps(full)
    res = bass_utils.run_bass_kernel_spmd(nc, in_maps, list(range(NCORES)))
    return np.concatenate([res.results[i]["out"] for i in range(NCORES)], axis=0).astype(np.float32)


# revision 42
# speedup vs baseline: 1.1132x; 1.1132x over previous
"""Trainium2 Bass kernel for the ASAP dual-branch GNN (GraphConv mean-aggr).

Strategy (data-parallel over graphs, 32 graphs per NeuronCore):
  * Host folds each graph's edge list into a dense normalized adjacency
    An[src, dst] = count(src->dst) / max(deg_dst, 1) in bf16 (one bincount
    over all edges), so on-device the GraphConv layers are pure dense
    matmuls: h = relu( wrel^T (x An) + wroot^T x + brel ).
  * Graphs processed in pairs packed side-by-side in the free axis, so the
    shared-weight root matmuls stream 400 columns at once; each pair's x
    arrives via the GpSimd queue and An via the Sync queue (one DMA each
    covering both branches), keeping per-queue issue cost off the PE path.
  * An split into two 100-row src halves serving as the PE moving operand
    with the node-major ys = x@wrel halves as stationary weights.
  * Engine balance: PE matmuls; ACT pair-wide relu+bias and half the
    psum->bf16 copies; DVE the other copies + per-graph pooled readout via
    free-axis tensor_reduce interleaved so casts stay ahead of the posts
    the PE waits on. MLP weights load on the Scalar queue; exp/ln ACT
    tables are pre-warmed so the log_softmax tail skips the table loads.
  * Small MLP head + log_softmax computed per-core in f32; no collectives.
Host side only does sharding/layout: adjacency histogram + degree fold,
transposes, dtype casts, blob packing, and constant folding of the
mean-pool 1/200 into lin1_w.
"""

import os
import sys

import numpy as np

if "/opt/trn_rl_repo" not in sys.path:
    sys.path.insert(0, "/opt/trn_rl_repo")

B, N, EPG = 256, 200, 3200
F, H, C = 200, 128, 2
NCORES = 8
GPC = B // NCORES  # graphs per core
NPAIR = GPC // 2
NQ = 100  # src-half width

_CACHE = {}


def _f32(x):
    return np.ascontiguousarray(x, dtype=np.float32)


def _build(gpc=GPC, repeat=1):
    import concourse.bass as bass
    import concourse.tile as tile
    from concourse import bacc, mybir

    dt = mybir.dt
    AF = mybir.ActivationFunctionType
    OP = mybir.AluOpType
    assert gpc % 2 == 0
    npair = gpc // 2

    nc = bacc.Bacc("TRN2", target_bir_lowering=False, debug=False)

    # ---- DRAM I/O (graph pairs packed in the free axis, branch inside) ----
    xta_d = nc.dram_tensor("xta", [npair, 128, 2, 2, N], dt.bfloat16, kind="ExternalInput").ap()
    xtb_d = nc.dram_tensor("xtb", [npair, F - 128, 2, 2, N], dt.bfloat16, kind="ExternalInput").ap()
    an_d = nc.dram_tensor("an", [npair, NQ, 2, 2, 2, N], dt.bfloat16, kind="ExternalInput").ap()
    wa_d = nc.dram_tensor("wa", [128, 2, 2, H], dt.bfloat16, kind="ExternalInput").ap()
    wb_d = nc.dram_tensor("wb", [F - 128, 2, 2, H], dt.bfloat16, kind="ExternalInput").ap()
    w2_d = nc.dram_tensor("w2", [128, 2, 2, H], dt.bfloat16, kind="ExternalInput").ap()
    br_d = nc.dram_tensor("brel", [128, 2, 2, 1], dt.float32, kind="ExternalInput").ap()
    l1w_d = nc.dram_tensor("l1w", [128, 4, H], dt.float32, kind="ExternalInput").ap()
    l1b_d = nc.dram_tensor("l1b", [1, H], dt.float32, kind="ExternalInput").ap()
    l2w_d = nc.dram_tensor("l2w", [H, H // 2], dt.float32, kind="ExternalInput").ap()
    l2b_d = nc.dram_tensor("l2b", [1, H // 2], dt.float32, kind="ExternalInput").ap()
    l3w_d = nc.dram_tensor("l3w", [H // 2, C], dt.float32, kind="ExternalInput").ap()
    l3b_d = nc.dram_tensor("l3b", [1, C], dt.float32, kind="ExternalInput").ap()
    out_d = nc.dram_tensor("out", [gpc, C], dt.float32, kind="ExternalOutput").ap()

    # ---- inline constants ----
    onesrow_f_d = nc.inline_tensor(_f32(np.ones((1, 128), np.float32)), "onesrowf").ap()
    ident_d = nc.inline_tensor(_f32(np.eye(128, dtype=np.float32)), "identf").ap()

    with tile.TileContext(nc) as tc:
        with (
            tc.tile_pool(name="cpool", bufs=1) as cpool,
            tc.tile_pool(name="xpool", bufs=4) as xpool,
            tc.tile_pool(name="apool", bufs=4) as apool,
            tc.tile_pool(name="hpool", bufs=6) as hpool,
            tc.tile_pool(name="spool", bufs=12) as spool,
            tc.tile_pool(name="rpool", bufs=4) as rpool,
            tc.tile_pool(name="psC", bufs=4, space="PSUM") as psC,
            tc.tile_pool(name="psD", bufs=4, space="PSUM") as psD,
        ):
            # ---- load conv weights (sync queue, first so conv starts ASAP) ----
            wa = cpool.tile([128, 2, 2, H], dt.bfloat16)
            nc.sync.dma_start(out=wa[:], in_=wa_d[:])
            wb = cpool.tile([F - 128, 2, 2, H], dt.bfloat16)
            nc.sync.dma_start(out=wb[:], in_=wb_d[:])
            w2 = cpool.tile([128, 2, 2, H], dt.bfloat16)
            nc.sync.dma_start(out=w2[:], in_=w2_d[:])
            brl = cpool.tile([128, 2, 2, 1], dt.float32)
            nc.sync.dma_start(out=brl[:], in_=br_d[:])
            # ---- MLP weights on the scalar queue (off the conv-critical path) ----
            onesrow_f = cpool.tile([1, 128], dt.float32)
            nc.scalar.dma_start(out=onesrow_f[:], in_=onesrow_f_d[:])
            ident = cpool.tile([128, 128], dt.float32)
            nc.scalar.dma_start(out=ident[:], in_=ident_d[:])
            l1w = cpool.tile([128, 4, H], dt.float32)
            nc.scalar.dma_start(out=l1w[:], in_=l1w_d[:])
            l1b = cpool.tile([1, H], dt.float32)
            nc.scalar.dma_start(out=l1b[:], in_=l1b_d[:])
            l2w = cpool.tile([H, H // 2], dt.float32)
            nc.scalar.dma_start(out=l2w[:], in_=l2w_d[:])
            l2b = cpool.tile([1, H // 2], dt.float32)
            nc.scalar.dma_start(out=l2b[:], in_=l2b_d[:])
            l3w = cpool.tile([H // 2, C], dt.float32)
            nc.scalar.dma_start(out=l3w[:], in_=l3w_d[:])
            l3b = cpool.tile([1, C], dt.float32)
            nc.scalar.dma_start(out=l3b[:], in_=l3b_d[:])
            # preload exp/ln ACT tables so the log_softmax tail doesn't pay them
            warm = rpool.tile([1, 1], dt.float32, tag="warm")
            nc.scalar.activation(warm[:], onesrow_f[:, 0:1], AF.Exp, bias=0.0, scale=1.0)
            nc.scalar.activation(warm[:], warm[:], AF.Ln, bias=0.0, scale=1.0)

            pooled = [
                [cpool.tile([128, gpc], dt.float32, tag=f"pool{b}{l}", name=f"pooled{b}{l}") for l in range(2)]
                for b in range(2)
            ]

            def conv_pair(b, p, xta, xtb, A):
                # Layer 1: ys = x @ wrel (node-major halves), per graph
                yss = []
                for g in range(2):
                    yr = psC.tile([NQ, 2, H], dt.float32, tag="yr", name=f"yr{b}{p}{g}")
                    for m in range(2):
                        nc.tensor.matmul(
                            yr[:, m, :], lhsT=xta[:, g, m * NQ : (m + 1) * NQ], rhs=wa[:, b, 0, :],
                            start=True, stop=False,
                        )
                        nc.tensor.matmul(
                            yr[:, m, :], lhsT=xtb[:, g, m * NQ : (m + 1) * NQ], rhs=wb[:, b, 0, :],
                            start=False, stop=True,
                        )
                    ys = spool.tile([NQ, 2, H], dt.bfloat16, tag="ys", name=f"ys{b}{p}{g}")
                    if g == 0:
                        nc.vector.tensor_copy(out=ys[:], in_=yr[:])
                    else:
                        nc.scalar.copy(out=ys[:], in_=yr[:])
                    yss.append(ys)

                hp = psD.tile([128, 2, N], dt.float32, tag="hD", name=f"hp{b}{p}")
                nc.tensor.matmul(hp[:, :, :], lhsT=wa[:, b, 1, :], rhs=xta[:, :, :], start=True, stop=False)
                nc.tensor.matmul(hp[:, :, :], lhsT=wb[:, b, 1, :], rhs=xtb[:, :, :], start=False, stop=False)
                for g in range(2):
                    nc.tensor.matmul(hp[:, g, :], lhsT=yss[g][:, 0, :], rhs=A[:, g, 0, :], start=False, stop=False)
                    nc.tensor.matmul(
                        hp[:, g, :], lhsT=yss[g][:, 1, :], rhs=A[:, g, 1, :],
                        start=False, stop=(g == 1),
                    )
                h1 = hpool.tile([128, 2, N], dt.bfloat16, tag="h1", name=f"h1{b}{p}")
                nc.scalar.activation(h1[:, :, :], hp[:, :, :], AF.Relu, bias=brl[:, b, 0, :], scale=1.0)
                nc.vector.tensor_reduce(
                    out=pooled[b][0][:, 2 * p : 2 * p + 2], in_=h1[:, :, :],
                    axis=mybir.AxisListType.X, op=OP.add,
                )

                # Layer 2
                zss = []
                for g in range(2):
                    zr = psC.tile([NQ, 2, H], dt.float32, tag="yr", name=f"zr{b}{p}{g}")
                    for m in range(2):
                        nc.tensor.matmul(
                            zr[:, m, :], lhsT=h1[:, g, m * NQ : (m + 1) * NQ], rhs=w2[:, b, 0, :],
                            start=True, stop=True,
                        )
                    zs = spool.tile([NQ, 2, H], dt.bfloat16, tag="ys", name=f"zs{b}{p}{g}")
                    if g == 0:
                        nc.vector.tensor_copy(out=zs[:], in_=zr[:])
                    else:
                        nc.scalar.copy(out=zs[:], in_=zr[:])
                    zss.append(zs)

                gp = psD.tile([128, 2, N], dt.float32, tag="hD", name=f"gp{b}{p}")
                nc.tensor.matmul(gp[:, :, :], lhsT=w2[:, b, 1, :], rhs=h1[:, :, :], start=True, stop=False)
                for g in range(2):
                    nc.tensor.matmul(gp[:, g, :], lhsT=zss[g][:, 0, :], rhs=A[:, g, 0, :], start=False, stop=False)
                    nc.tensor.matmul(
                        gp[:, g, :], lhsT=zss[g][:, 1, :], rhs=A[:, g, 1, :],
                        start=False, stop=(g == 1),
                    )
                g1 = hpool.tile([128, 2, N], dt.bfloat16, tag="g1", name=f"g1{b}{p}")
                nc.scalar.activation(g1[:, :, :], gp[:, :, :], AF.Relu, bias=brl[:, b, 1, :], scale=1.0)
                nc.vector.tensor_reduce(
                    out=pooled[b][1][:, 2 * p : 2 * p + 2], in_=g1[:, :, :],
                    axis=mybir.AxisListType.X, op=OP.add,
                )

            # ---- main loop: graph pairs ----
            for _rep in range(repeat):
                for p in range(npair):
                    xtap = xpool.tile([128, 2, 2, N], dt.bfloat16, tag="xta", name=f"xta{p}")
                    nc.gpsimd.dma_start(out=xtap[:], in_=xta_d[p])
                    xtbp = xpool.tile([F - 128, 2, 2, N], dt.bfloat16, tag="xtb", name=f"xtb{p}")
                    nc.gpsimd.dma_start(out=xtbp[:], in_=xtb_d[p])
                    Ap = apool.tile([NQ, 2, 2, 2, N], dt.bfloat16, tag="A", name=f"A{p}")
                    nc.sync.dma_start(out=Ap[:], in_=an_d[p])
                    for b in range(2):
                        conv_pair(b, p, xtap[:, b], xtbp[:, b], Ap[:, b])

            # ---- MLP head (f32) ----
            z1p = psD.tile([gpc, H], dt.float32, tag="hD")
            order = [pooled[0][0], pooled[0][1], pooled[1][0], pooled[1][1]]
            for k in range(4):
                nc.tensor.matmul(z1p[:], lhsT=order[k][:], rhs=l1w[:, k, :], start=(k == 0), stop=False)
            nc.tensor.matmul(z1p[:], lhsT=onesrow_f[:, 0:gpc], rhs=l1b[:], start=False, stop=True)
            z1s = rpool.tile([gpc, H], dt.float32, tag="z1s")
            nc.scalar.activation(z1s[:], z1p[:], AF.Relu, bias=0.0, scale=1.0)

            z1tp = psD.tile([H, gpc], dt.float32, tag="hD")
            nc.tensor.transpose(out=z1tp[:], in_=z1s[:], identity=ident[0:gpc, 0:gpc])
            z1t = rpool.tile([H, gpc], dt.float32, tag="z1t")
            nc.vector.tensor_copy(out=z1t[:], in_=z1tp[:])

            z2p = psD.tile([gpc, H // 2], dt.float32, tag="hD")
            nc.tensor.matmul(z2p[:], lhsT=z1t[:], rhs=l2w[:], start=True, stop=False)
            nc.tensor.matmul(z2p[:], lhsT=onesrow_f[:, 0:gpc], rhs=l2b[:], start=False, stop=True)
            z2s = rpool.tile([gpc, H // 2], dt.float32, tag="z2s")
            nc.scalar.activation(z2s[:], z2p[:], AF.Relu, bias=0.0, scale=1.0)

            z2tp = psD.tile([H // 2, gpc], dt.float32, tag="hD")
            nc.tensor.transpose(out=z2tp[:], in_=z2s[:], identity=ident[0:gpc, 0:gpc])
            z2t = rpool.tile([H // 2, gpc], dt.float32, tag="z2t")
            nc.vector.tensor_copy(out=z2t[:], in_=z2tp[:])

            z3p = psD.tile([gpc, C], dt.float32, tag="hD")
            nc.tensor.matmul(z3p[:], lhsT=z2t[:], rhs=l3w[:], start=True, stop=False)
            nc.tensor.matmul(z3p[:], lhsT=onesrow_f[:, 0:gpc], rhs=l3b[:], start=False, stop=True)

            m = rpool.tile([gpc, 1], dt.float32, tag="lsm")
            nc.vector.tensor_reduce(out=m[:], in_=z3p[:], axis=mybir.AxisListType.X, op=OP.max)
            negm = rpool.tile([gpc, 1], dt.float32, tag="lsnm")
            nc.vector.tensor_scalar(negm[:], m[:], -1.0, None, OP.mult)
            esc = rpool.tile([gpc, C], dt.float32, tag="lse")
            sume = rpool.tile([gpc, 1], dt.float32, tag="lssum")
            nc.scalar.activation(esc[:], z3p[:], AF.Exp, bias=negm[:], scale=1.0, accum_out=sume[:])
            lse = rpool.tile([gpc, 1], dt.float32, tag="lsl")
            nc.scalar.activation(lse[:], sume[:], AF.Ln, bias=0.0, scale=1.0)
            outv = rpool.tile([gpc, C], dt.float32, tag="outv")
            nc.vector.tensor_scalar(outv[:], z3p[:], negm[:], lse[:], OP.add, OP.subtract)
            nc.sync.dma_start(out=out_d[:], in_=outv[:])

    nc.compile()
    return nc


def _prep_inputs(sc_x, fc_x, sc_edge_index, fc_edge_index,
                 sc1_wrel, sc1_brel, sc1_wroot, sc2_wrel, sc2_brel, sc2_wroot,
                 fc1_wrel, fc1_brel, fc1_wroot, fc2_wrel, fc2_brel, fc2_wroot,
                 lin1_w, lin1_b, lin2_w, lin2_b, lin3_w, lin3_b, batch=None):
    import ml_dtypes

    bf = ml_dtypes.bfloat16

    def prep_x(x):
        # [B*N, F] -> [B/2, F, 2, N] (graph pairs packed in free axis)
        return np.asarray(x, np.float32).reshape(B // 2, 2, N, F).transpose(0, 3, 1, 2)

    def prep_A(ei):
        # dense normalized adjacency: An[g, src, dst] = count / max(deg_dst, 1)
        ei = np.asarray(ei).astype(np.int64)
        gid = np.arange(B * EPG, dtype=np.int64) // EPG
        src = ei[0] - gid * N
        dst = ei[1] - gid * N
        flat = (gid * N + src) * N + dst
        cnt = np.bincount(flat, minlength=B * N * N).astype(np.float32).reshape(B, N, N)
        deg = cnt.sum(axis=1)  # in-degree per dst
        An = cnt / np.maximum(deg, 1.0)[:, None, :]
        # [B, N(src), N(dst)] -> [B/2, 2(graph), 2(src half), 100, 200]
        return An.reshape(B // 2, 2, 2, NQ, N)

    # [B/2, F, 2(branch), 2(graph), N]
    xt = np.stack([prep_x(sc_x), prep_x(fc_x)]).transpose(1, 2, 0, 3, 4)
    xta = np.ascontiguousarray(xt[:, 0:128]).astype(bf)
    xtb = np.ascontiguousarray(xt[:, 128:F]).astype(bf)
    # [B/2, 100, 2(branch), 2(graph), 2(half), 200]
    an = np.ascontiguousarray(
        np.stack([prep_A(sc_edge_index), prep_A(fc_edge_index)]).transpose(1, 4, 0, 2, 3, 5)
    ).astype(bf)

    wa = np.stack([
        np.stack([np.asarray(sc1_wrel)[:128], np.asarray(sc1_wroot)[:128]]),
        np.stack([np.asarray(fc1_wrel)[:128], np.asarray(fc1_wroot)[:128]]),
    ]).transpose(2, 0, 1, 3).astype(bf)  # [128, 2, 2, H] (partition-major)
    wb = np.stack([
        np.stack([np.asarray(sc1_wrel)[128:], np.asarray(sc1_wroot)[128:]]),
        np.stack([np.asarray(fc1_wrel)[128:], np.asarray(fc1_wroot)[128:]]),
    ]).transpose(2, 0, 1, 3).astype(bf)  # [72, 2, 2, H]
    w2 = np.stack([
        np.stack([np.asarray(sc2_wrel), np.asarray(sc2_wroot)]),
        np.stack([np.asarray(fc2_wrel), np.asarray(fc2_wroot)]),
    ]).transpose(2, 0, 1, 3).astype(bf)  # [128, 2, 2, H]
    brel = np.stack([
        np.stack([np.asarray(sc1_brel), np.asarray(sc2_brel)]),
        np.stack([np.asarray(fc1_brel), np.asarray(fc2_brel)]),
    ]).astype(np.float32).transpose(2, 0, 1)[:, :, :, None]  # [128, 2, 2, 1]

    l1w = np.asarray(lin1_w, np.float32).copy()
    l1w[:256] *= 1.0 / N  # fold mean-pool divisor for the SC branch readouts
    l1w = np.ascontiguousarray(l1w.reshape(4, 128, H).transpose(1, 0, 2))  # [128, 4, H]

    return dict(
        xta=xta, xtb=xtb, an=an, wa=_c(wa), wb=_c(wb), w2=_c(w2), brel=_c(brel),
        l1w=l1w, l1b=_f32(lin1_b)[None, :], l2w=_f32(lin2_w), l2b=_f32(lin2_b)[None, :],
        l3w=_f32(lin3_w), l3b=_f32(lin3_b)[None, :],
    )


def _c(x):
    return np.ascontiguousarray(x)


def _make_in_maps(full):
    in_maps = []
    for c in range(NCORES):
        ps = slice(c * NPAIR, (c + 1) * NPAIR)
        m = dict(full)
        m["xta"] = np.ascontiguousarray(full["xta"][ps])
        m["xtb"] = np.ascontiguousarray(full["xtb"][ps])
        m["an"] = np.ascontiguousarray(full["an"][ps])
        in_maps.append(m)
    return in_maps


def kernel(**inputs):
    from concourse import bass_utils

    if "nc" not in _CACHE:
        _CACHE["nc"] = _build()
    nc = _CACHE["nc"]

    full = _prep_inputs(**inputs)
    in_maps = _make_in_maps(full)
    res = bass_utils.run_bass_kernel_spmd(nc, in_maps, list(range(NCORES)))
    return np.concatenate([res.results[i]["out"] for i in range(NCORES)], axis=0).astype(np.float32)
